# revision 1
# baseline (speedup 1.0000x reference)
"""Trainium2 Bass kernel for per-edge dot products (DGL u_dot_v).

score[e] = sum_d h[src[e], d] * h[dst[e], d]   for 640K edges, 10K nodes, D=128.

Strategy (8 NeuronCores, data-parallel over edges):
  - Each core gets 80K edges; h stays in HBM, replicated per core.
  - Per tile of 4096 edges: two HBM-source `dma_gather`s pull h rows as
    contiguous descriptors into SBUF as [128 edges, 32, 128 features]
    (edge i -> partition i%128, slot i//128), spread across SWDGE queues.
  - VectorE: one elementwise multiply + one free-dim `tensor_reduce` per tile
    produce [128, 32] fp32 scores.
  - One contiguous DMA writes [128, 625] scores out; the host inverts the
    (partition, chunk) interleave with a transpose-reshape.
"""

import sys

import numpy as np

for _p in ("/opt/trn_rl_repo", "/opt/pypackages"):
    if _p not in sys.path:
        sys.path.append(_p)

import concourse.mybir as mybir  # noqa: E402
import concourse.tile as tile  # noqa: E402
from concourse import bacc  # noqa: E402
from concourse.bass_utils import run_bass_kernel_spmd  # noqa: E402

N_NODES = 10000
D_FEAT = 128
N_EDGES = 640000
N_CORES = 8
E_PER = N_EDGES // N_CORES  # 80000
TILE_E = 4096  # edges per gather tile

# Gather precision: fp32 h rows are exact; fp16 halves gather traffic at
# ~1.2e-4 scale-relative error (products/accumulation stay fp32 on DVE).
GATHER_DTYPE = "f32"
N_QUEUES = 4  # SWDGE queues to spread gathers over (1..4)

_BUILT = {}


def _edge_tiles(e_per):
    tiles = []
    s = 0
    while s < e_per:
        t = min(TILE_E, e_per - s)
        assert t % 128 == 0
        tiles.append((s, t))
        s += t
    return tiles


def build(e_per=E_PER, reps=1, gdt=None, n_queues=None):
    """Build + compile the per-core Bass program (cached).

    reps > 1 repeats the whole compute (for wall-clock differencing in the
    bench harness); output is identical for every rep."""
    gdt = gdt or GATHER_DTYPE
    n_queues = n_queues or N_QUEUES
    key = (e_per, reps, gdt, n_queues)
    if key in _BUILT:
        return _BUILT[key]

    i16 = mybir.dt.int16
    f32 = mybir.dt.float32
    gdtype = f32 if gdt == "f32" else mybir.dt.float16

    nc = bacc.Bacc(
        "TRN2", target_bir_lowering=False, debug=False, num_swdge_queues=n_queues
    )

    h_d = nc.dram_tensor("h", [N_NODES, D_FEAT], gdtype, kind="ExternalInput")
    srcw_d = nc.dram_tensor("srcw", [128, e_per // 16], i16, kind="ExternalInput")
    dstw_d = nc.dram_tensor("dstw", [128, e_per // 16], i16, kind="ExternalInput")
    out_d = nc.dram_tensor("scores", [128, e_per // 128], f32, kind="ExternalOutput")

    with tile.TileContext(nc) as tc:
        with (
            tc.tile_pool(name="const", bufs=1) as constp,
            tc.tile_pool(name="gather", bufs=3) as gpool,
            tc.tile_pool(name="prod", bufs=2) as ppool,
            tc.tile_pool(name="outp", bufs=1) as outp,
        ):
            srcw = constp.tile([128, e_per // 16], i16)
            dstw = constp.tile([128, e_per // 16], i16)
            scores = outp.tile([128, e_per // 128], f32)

            nc.sync.dma_start(srcw[:], srcw_d[:])
            nc.sync.dma_start(dstw[:], dstw_d[:])

            q = 0
            for start, t in _edge_tiles(e_per) * reps:
                nchunk = t // 128
                hu = gpool.tile([128, nchunk, D_FEAT], gdtype, tag="hu")
                hv = gpool.tile([128, nchunk, D_FEAT], gdtype, tag="hv")
                for dst_t, idx_t in ((hu, srcw), (hv, dstw)):
                    nc.gpsimd.dma_gather(
                        dst_t[:],
                        h_d[:],
                        idx_t[:, start // 16 : (start + t) // 16],
                        num_idxs=t,
                        num_idxs_reg=t,
                        elem_size=D_FEAT,
                        single_packet=False,
                        queue_num=q % n_queues,
                    )
                    q += 1
                prod = ppool.tile([128, nchunk, D_FEAT], f32)
                nc.vector.tensor_mul(prod[:], hu[:], hv[:])
                nc.vector.tensor_reduce(
                    scores[:, start // 128 : start // 128 + nchunk],
                    prod[:],
                    axis=mybir.AxisListType.X,
                    op=mybir.AluOpType.add,
                )

            nc.sync.dma_start(out_d[:], scores[:])

    nc.compile()
    _BUILT[key] = nc
    return nc


def wrap_idx(ix):
    """Edge indices [E_c] -> int16 [128, E_c/16]: slot j read from
    (partition j%16, col j//16), replicated across the 8 GPSIMD core groups."""
    w = ix.astype(np.int16).reshape(-1, 16).T  # [16, E_c/16]
    return np.ascontiguousarray(np.tile(w, (8, 1)))


# ---------------------------------------------------------------------------
# Paired variant: sort edges by src so pairs of edges share one src-row gather
# (hu descriptors halve: 160K -> 129K rows gathered per core).
# Device slot layout: blocks of 256 slots = 128 pairs; pair i -> slots
# (i//128)*256 + i%128 + {0, 128}, so both edges of a pair sit on partition
# i%128, matching the hu2 gather interleave (row i -> partition i%128).
# ---------------------------------------------------------------------------

E2_PER = 86016  # padded device slots per core (multiple of 256, >= worst pad)


def build_paired(e2=E2_PER, reps=1, n_queues=None):
    n_queues = n_queues or N_QUEUES
    key = ("paired", e2, reps, n_queues)
    if key in _BUILT:
        return _BUILT[key]

    i16 = mybir.dt.int16
    f32 = mybir.dt.float32

    nc = bacc.Bacc(
        "TRN2", target_bir_lowering=False, debug=False, num_swdge_queues=n_queues
    )

    h_d = nc.dram_tensor("h", [N_NODES, D_FEAT], f32, kind="ExternalInput")
    srcw_d = nc.dram_tensor("srcw", [128, e2 // 32], i16, kind="ExternalInput")
    dstw_d = nc.dram_tensor("dstw", [128, e2 // 16], i16, kind="ExternalInput")
    out_d = nc.dram_tensor("scores", [128, e2 // 128], f32, kind="ExternalOutput")

    with tile.TileContext(nc) as tc:
        with (
            tc.tile_pool(name="const", bufs=1) as constp,
            tc.tile_pool(name="gather", bufs=3) as gpool,
            tc.tile_pool(name="prod", bufs=2) as ppool,
            tc.tile_pool(name="outp", bufs=4) as outp,
        ):
            srcw = constp.tile([128, e2 // 32], i16)
            dstw = constp.tile([128, e2 // 16], i16)

            nc.sync.dma_start(srcw[:], srcw_d[:])
            nc.sync.dma_start(dstw[:], dstw_d[:])

            q = 0
            for start, t in _edge_tiles(e2) * reps:
                nb = t // 256  # pair-blocks in this tile
                hu2 = gpool.tile([128, nb, D_FEAT], f32, tag="hu2")
                hv3 = gpool.tile([128, t // 128, D_FEAT], f32, tag="hv3")
                prod3 = ppool.tile([128, t // 128, D_FEAT], f32)
                hv4 = hv3[:].rearrange("p (b r) f -> p b r f", r=2)
                prod4 = prod3[:].rearrange("p (b r) f -> p b r f", r=2)
                for hf in range(2):
                    p0 = start // 2 + hf * (t // 4)
                    nc.gpsimd.dma_gather(
                        hu2[:, hf * nb // 2 : (hf + 1) * nb // 2, :],
                        h_d[:],
                        srcw[:, p0 // 16 : (p0 + t // 4) // 16],
                        num_idxs=t // 4,
                        num_idxs_reg=t // 4,
                        elem_size=D_FEAT,
                        single_packet=False,
                        queue_num=q % n_queues,
                    )
                    q += 1
                    h0 = hf * (t // 2)
                    nc.gpsimd.dma_gather(
                        hv3[:, hf * (t // 256) : (hf + 1) * (t // 256), :],
                        h_d[:],
                        dstw[:, (start + h0) // 16 : (start + h0 + t // 2) // 16],
                        num_idxs=t // 2,
                        num_idxs_reg=t // 2,
                        elem_size=D_FEAT,
                        single_packet=False,
                        queue_num=q % n_queues,
                    )
                    q += 1
                    bs = slice(hf * nb // 2, (hf + 1) * nb // 2)
                    nc.vector.tensor_mul(
                        prod4[:, bs, 0, :], hu2[:, bs, :], hv4[:, bs, 0, :]
                    )
                    nc.vector.tensor_mul(
                        prod4[:, bs, 1, :], hu2[:, bs, :], hv4[:, bs, 1, :]
                    )
                sc = outp.tile([128, t // 128], f32, tag="sc")
                nc.vector.tensor_reduce(
                    sc[:],
                    prod3[:],
                    axis=mybir.AxisListType.X,
                    op=mybir.AluOpType.add,
                )
                nc.sync.dma_start(
                    out_d[:, start // 128 : start // 128 + t // 128], sc[:]
                )

    nc.compile()
    _BUILT[key] = nc
    return nc


def prep_paired(s, d, e2=E2_PER):
    """Sort a core's edges by src, pad equal-src runs to even length, and lay
    pairs out in the device block order. Returns (hu_idx [e2/2], hv_idx [e2],
    ed_map [e2] original-edge-or--1) or None if padding overflows e2."""
    n = len(s)
    order = np.argsort(s, kind="stable")
    ss, dd = s[order], d[order]
    change = np.flatnonzero(np.diff(ss)) + 1
    starts = np.concatenate(([0], change))
    ends = np.concatenate((change, [n]))
    lens = ends - starts
    odd = (lens % 2).astype(bool)
    if n + int(odd.sum()) > e2:
        return None
    pads_before = np.concatenate(([0], np.cumsum(odd)[:-1]))
    new_pos = np.arange(n) + np.repeat(pads_before, lens)
    psrc = np.zeros(e2, np.int64)
    pdst = np.zeros(e2, np.int64)
    pedge = np.full(e2, -1, np.int64)
    psrc[new_pos] = ss
    pdst[new_pos] = dd
    pedge[new_pos] = order
    pad_slots = (ends + pads_before)[odd]
    psrc[pad_slots] = ss[ends[odd] - 1]
    j = np.arange(e2)
    ps = 2 * ((j // 256) * 128 + (j % 128)) + (j % 256) // 128
    return psrc[0::2], pdst[ps], pedge[ps]


def _kernel_flat(h, src, dst):
    """Unpaired path: one gather per edge endpoint."""
    nc = build(E_PER)
    in_maps = []
    for k in range(N_CORES):
        sl = slice(k * E_PER, (k + 1) * E_PER)
        in_maps.append(
            {"h": h, "srcw": wrap_idx(src[sl]), "dstw": wrap_idx(dst[sl])}
        )
    res = run_bass_kernel_spmd(nc, in_maps, list(range(N_CORES)))
    parts = []
    for k in range(N_CORES):
        sc = res.results[k]["scores"]  # [128, E_PER/128]; edge j at [j%128, j//128]
        parts.append(sc.T.reshape(-1))
    return np.concatenate(parts).astype(np.float32).reshape(N_EDGES, 1)


def kernel(h, src, dst):
    np_gdt = np.float32 if GATHER_DTYPE == "f32" else np.float16
    h = np.ascontiguousarray(np.asarray(h, dtype=np.float32).astype(np_gdt))
    src = np.asarray(src).astype(np.int64)
    dst = np.asarray(dst).astype(np.int64)

    preps = []
    for k in range(N_CORES):
        sl = slice(k * E_PER, (k + 1) * E_PER)
        preps.append(prep_paired(src[sl], dst[sl]))
    if any(p is None for p in preps):
        return _kernel_flat(h, src, dst)

    nc = build_paired(E2_PER)
    in_maps = []
    for hu_idx, hv_idx, _ in preps:
        in_maps.append(
            {"h": h, "srcw": wrap_idx(hu_idx), "dstw": wrap_idx(hv_idx)}
        )
    res = run_bass_kernel_spmd(nc, in_maps, list(range(N_CORES)))

    out = np.empty(N_EDGES, np.float32)
    for k in range(N_CORES):
        sc = res.results[k]["scores"]  # [128, E2/128]; device slot j at [j%128, j//128]
        flat = sc.T.reshape(-1)
        ed_map = preps[k][2]
        valid = ed_map >= 0
        out_local = np.empty(E_PER, np.float32)
        out_local[ed_map[valid]] = flat[valid]
        out[k * E_PER : (k + 1) * E_PER] = out_local
    return out.reshape(N_EDGES, 1)



# revision 12
# speedup vs baseline: 52.4079x; 52.4079x over previous
"""Trainium2 Bass kernel for per-edge dot products (DGL u_dot_v).

score[e] = sum_d h[src[e], d] * h[dst[e], d]   for 640K edges, 10K nodes, D=128.

Strategy (8 NeuronCores, SPMD, one-hot matmul gather — no per-edge DMA):
  - Nodes split into 79 blocks of 128. Edges bucketed by
    (src block, dst block). Src blocks ("rows") are sharded across cores
    (<=10 rows/core); each core runs a STATIC cartesian schedule of
    790 tiles: tile t covers bucket (local row t//79, dst block t%79),
    holding up to 128 edges. ~1% of buckets exceed 128 edges; overflow
    goes to 16 fixed "spill" tiles whose h blocks ship as data.
  - Per tile, the edge membership is encoded host-side as two fp8 one-hot
    matrices [128 nodes x 128 edges]. The gathers are PE matmuls:
    hu = onehot_src.T @ h_row (stationary fp8 one-hot, moving bf16 h
    block resident in SBUF), hv likewise with the full-h replica.
    A fused DVE tensor_tensor_reduce computes the 128 dot products.
  - Program structure is input-independent: all data-dependence lives in
    the shipped arrays, so one compile serves any input (SPMD-uniform).
"""

import sys

import numpy as np

for _p in ("/opt/trn_rl_repo", "/opt/pypackages"):
    if _p not in sys.path:
        sys.path.append(_p)

import ml_dtypes  # noqa: E402

import concourse.mybir as mybir  # noqa: E402
import concourse.tile as tile  # noqa: E402
from concourse import bacc, bass  # noqa: E402
from concourse.bass_utils import run_bass_kernel_spmd  # noqa: E402

N_NODES = 10000
D_FEAT = 128
N_EDGES = 640000
N_CORES = 8
P = 128
NBLK = (N_NODES + P - 1) // P  # 79 node blocks
NR = 10  # src rows per core (7 cores x 10 + 1 core x 9)
TBASE = NR * NBLK  # 790 schedule tiles
SPILL = 18  # spill tiles for buckets > 128 edges
T_TILES = TBASE + SPILL  # 808
CH = 64  # tiles per one-hot DMA chunk (1 MiB per side)
G = 4  # tiles per PSUM group (gather batch / reduce-matmul granularity)
NG = T_TILES // G  # 202 groups
DRAIN = 3  # groups per PSUM score tile (partitions 0/32/64; quadrant 3 is unusable)
ND = (NG + DRAIN - 1) // DRAIN  # 51 drains
LAG = 2  # groups between DVE mult and PE reduce (software pipelining)

_F8 = ml_dtypes.float8_e4m3
_BF16 = ml_dtypes.bfloat16

_BUILT = {}


def build_v4(reps=1):
    """Compile the SPMD per-core program (cached; input-independent).

    v6 pipeline per 4-tile group:
      PE:  huT[d,e] = h_i @ ohs   hvT[d,e] = h_j @ ohd   (transposed gathers,
           stationary = bf16 h block, moving = fp8 one-hot)
      ACT: hvT batch-copy PSUM -> SBUF bf16
      DVE: prodT = huT(PSUM) * hvT(SBUF)  -> SBUF bf16   (one op per group)
      PE:  scores[1, G*128] = ones.T @ prodT  (reduce over features; emitted
           LAG groups late so PE never stalls on DVE; lands at a 32-aligned
           PSUM partition, 4 groups per score tile)
      DVE: drain score tile PSUM -> SBUF;  DMA out per drain.
    """
    key = ("v6", reps)
    if key in _BUILT:
        return _BUILT[key]

    f32 = mybir.dt.float32
    bf16 = mybir.dt.bfloat16
    f8 = mybir.dt.float8e4

    nc = bacc.Bacc("TRN2", target_bir_lowering=False, debug=False)

    hfull_d = nc.dram_tensor("hfull", [P, NBLK, D_FEAT], bf16, kind="ExternalInput")
    hsrc_d = nc.dram_tensor("hsrc", [P, NR, D_FEAT], bf16, kind="ExternalInput")
    sps_d = nc.dram_tensor("sphs", [P, SPILL, D_FEAT], bf16, kind="ExternalInput")
    spd_d = nc.dram_tensor("sphd", [P, SPILL, D_FEAT], bf16, kind="ExternalInput")
    ohs_d = nc.dram_tensor("ohs", [P, T_TILES, P], f8, kind="ExternalInput")
    ohd_d = nc.dram_tensor("ohd", [P, T_TILES, P], f8, kind="ExternalInput")
    out_d = nc.dram_tensor("scores", [ND, P, G * P], f32, kind="ExternalOutput")

    with tile.TileContext(nc) as tc:
        with (
            tc.tile_pool(name="const", bufs=1) as constp,
            tc.tile_pool(name="oh", bufs=3) as ohp,
            tc.tile_pool(name="work", bufs=LAG + 2) as workp,
            tc.tile_pool(name="gps", bufs=2, space=bass.MemorySpace.PSUM) as gpsp,
            tc.tile_pool(name="rps", bufs=2, space=bass.MemorySpace.PSUM) as rpsp,
            tc.tile_pool(name="outp", bufs=3) as outp,
        ):
            hfull = constp.tile([P, NBLK, D_FEAT], bf16)
            hsrc = constp.tile([P, NR, D_FEAT], bf16)
            sps = constp.tile([P, SPILL, D_FEAT], bf16)
            spd = constp.tile([P, SPILL, D_FEAT], bf16)
            ones = constp.tile([P, 1], bf16)
            nc.vector.memset(ones[:], 1.0)

            def tile_operands(t):
                if t < TBASE:
                    return hsrc[:, t // NBLK, :], hfull[:, t % NBLK, :]
                return sps[:, t - TBASE, :], spd[:, t - TBASE, :]

            def body():
                nc.sync.dma_start(hfull[:], hfull_d[:])
                nc.sync.dma_start(hsrc[:], hsrc_d[:])
                nc.sync.dma_start(sps[:], sps_d[:])
                nc.sync.dma_start(spd[:], spd_d[:])

                state = {"red": None, "nred": 0, "pending": []}

                def flush_one():
                    g, prodT = state["pending"].pop(0)
                    q = g % DRAIN
                    if q == 0:
                        red = rpsp.tile([P, G * P], f32, tag="red")
                        state["red"] = red
                    red = state["red"]
                    nc.tensor.matmul(
                        red[32 * q : 32 * q + 1, :],
                        ones[:],
                        prodT[:].rearrange("p g e -> p (g e)"),
                    )
                    if q == DRAIN - 1 or g == NG - 1:
                        d = g // DRAIN
                        scs = outp.tile([P, G * P], f32, tag="scs")
                        nc.vector.tensor_copy(scs[:], red[:])
                        nc.sync.dma_start(out_d[d], scs[:])

                for c0 in range(0, T_TILES, CH):
                    n = min(CH, T_TILES - c0)
                    ohs_sb = ohp.tile([P, CH, P], f8, tag="ohs")
                    ohd_sb = ohp.tile([P, CH, P], f8, tag="ohd")
                    nc.sync.dma_start(ohs_sb[:, :n, :], ohs_d[:, c0 : c0 + n, :])
                    nc.sync.dma_start(ohd_sb[:, :n, :], ohd_d[:, c0 : c0 + n, :])
                    for g0 in range(0, n, G):
                        huT = gpsp.tile([P, G, P], f32, tag="huT")
                        hvT = gpsp.tile([P, G, P], f32, tag="hvT")
                        for k in range(G):
                            tt = g0 + k
                            rs, rd = tile_operands(c0 + tt)
                            nc.tensor.matmul(huT[:, k, :], rs, ohs_sb[:, tt, :])
                            nc.tensor.matmul(hvT[:, k, :], rd, ohd_sb[:, tt, :])
                        hvs = workp.tile([P, G, P], bf16, tag="hvs")
                        nc.scalar.activation(
                            hvs[:], hvT[:], mybir.ActivationFunctionType.Copy
                        )
                        prodT = workp.tile([P, G, P], bf16, tag="prodT")
                        nc.vector.tensor_tensor(
                            prodT[:], huT[:], hvs[:], op=mybir.AluOpType.mult
                        )
                        state["pending"].append(((c0 + g0) // G, prodT))
                        if len(state["pending"]) > LAG:
                            flush_one()
                while state["pending"]:
                    flush_one()

            if reps == 1:
                body()
            else:
                with tc.For_i(0, reps):
                    body()

    nc.compile()
    _BUILT[key] = nc
    return nc


def prep_v4(h, src, dst):
    """Host-side packing. Returns (in_maps, core, t, p) or None if the
    spill capacity is exceeded (caller falls back)."""
    E = len(src)
    i = src // P
    j = dst // P
    core = np.minimum(i // NR, N_CORES - 1)
    m = i - core * NR
    t = m * NBLK + j

    key = core * T_TILES + t
    order = np.argsort(key, kind="stable")
    ks = key[order]
    starts = np.r_[0, np.flatnonzero(np.diff(ks)) + 1]
    sizes = np.diff(np.r_[starts, len(ks)])
    rank = np.arange(E) - np.repeat(starts, sizes)
    p = np.empty(E, np.int64)
    p[order] = rank

    # spill: edges ranked >=128 in their bucket get spill tiles, one per
    # (core, bucket, 128-chunk), numbered per core in bucket order.
    ov = p >= P
    if np.any(ov):
        sk = key[ov] * 64 + (p[ov] - P) // P  # unique per spill tile
        uniq, inv = np.unique(sk, return_inverse=True)
        ucore = (uniq // 64) // T_TILES
        # spill index within core
        sidx = np.zeros(len(uniq), np.int64)
        for c in range(N_CORES):
            sel = ucore == c
            if sel.sum() > SPILL:
                return None
            sidx[sel] = np.arange(sel.sum())
        t = t.copy()
        t[ov] = TBASE + sidx[inv]
        p = p.copy()
        p[ov] = (p[ov] - P) % P

    # h layouts
    hpad = np.zeros((NBLK * P, D_FEAT), np.float32)
    hpad[:N_NODES] = h
    h_re = np.ascontiguousarray(
        hpad.reshape(NBLK, P, D_FEAT).transpose(1, 0, 2)
    ).astype(_BF16)  # [128, 79, 128]

    # one-hots: fp8 1.0 == 0x38 bit pattern
    ohs = np.zeros((N_CORES, P, T_TILES, P), np.uint8)
    ohd = np.zeros((N_CORES, P, T_TILES, P), np.uint8)
    ohs[core, src % P, t, p] = 0x38
    ohd[core, dst % P, t, p] = 0x38

    in_maps = []
    for c in range(N_CORES):
        rows = h_re[:, c * NR : c * NR + NR, :]
        if rows.shape[1] < NR:
            rows = np.concatenate(
                [rows, np.zeros((P, NR - rows.shape[1], D_FEAT), _BF16)], axis=1
            )
        sphs = np.zeros((P, SPILL, D_FEAT), _BF16)
        sphd = np.zeros((P, SPILL, D_FEAT), _BF16)
        csel = np.flatnonzero((core == c) & (t >= TBASE))
        if len(csel):
            s_ids = t[csel] - TBASE
            sphs[:, s_ids, :] = h_re[:, i[csel], :]
            sphd[:, s_ids, :] = h_re[:, j[csel], :]
        in_maps.append(
            {
                "hfull": h_re,
                "hsrc": np.ascontiguousarray(rows),
                "sphs": sphs,
                "sphd": sphd,
                "ohs": ohs[c].view(_F8),
                "ohd": ohd[c].view(_F8),
            }
        )
    return in_maps, core, t, p


def kernel(h, src, dst):
    h = np.asarray(h, dtype=np.float32)
    src = np.asarray(src).astype(np.int64)
    dst = np.asarray(dst).astype(np.int64)

    prep = prep_v4(h, src, dst)
    if prep is None:
        return _kernel_flat(h, src, dst)
    in_maps, core, t, p = prep

    nc = build_v4(reps=1)
    res = run_bass_kernel_spmd(nc, in_maps, list(range(N_CORES)))
    stacked = np.stack([np.asarray(res.results[c]["scores"]) for c in range(N_CORES)])
    g = t // G  # [8, ND, 128, G*128]; score(t, p) at [g//DRAIN, 32*(g%DRAIN), (t%G)*128+p]
    out = stacked[core, g // DRAIN, 32 * (g % DRAIN), (t % G) * P + p].astype(
        np.float32
    )
    return out.reshape(N_EDGES, 1)


# ---------------------------------------------------------------------------
# Fallback: SWDGE dma_gather path (slow but fully general) — used only if
# spill capacity is exceeded (pathological edge distributions).
# ---------------------------------------------------------------------------

TILE_E = 4096
E_PER = N_EDGES // N_CORES


def _edge_tiles(e_per):
    tiles = []
    s = 0
    while s < e_per:
        tl = min(TILE_E, e_per - s)
        tiles.append((s, tl))
        s += tl
    return tiles


def build_flat(e_per, reps=1):
    key = ("flat", e_per, reps)
    if key in _BUILT:
        return _BUILT[key]
    i16 = mybir.dt.int16
    f32 = mybir.dt.float32
    nc = bacc.Bacc("TRN2", target_bir_lowering=False, debug=False, num_swdge_queues=4)
    h_d = nc.dram_tensor("h", [N_NODES, D_FEAT], f32, kind="ExternalInput")
    srcw_d = nc.dram_tensor("srcw", [128, e_per // 16], i16, kind="ExternalInput")
    dstw_d = nc.dram_tensor("dstw", [128, e_per // 16], i16, kind="ExternalInput")
    out_d = nc.dram_tensor("scores", [128, e_per // 128], f32, kind="ExternalOutput")
    with tile.TileContext(nc) as tc:
        with (
            tc.tile_pool(name="const", bufs=1) as constp,
            tc.tile_pool(name="gather", bufs=3) as gpool,
            tc.tile_pool(name="prod", bufs=2) as ppool,
            tc.tile_pool(name="outp", bufs=1) as outp,
        ):
            srcw = constp.tile([128, e_per // 16], i16)
            dstw = constp.tile([128, e_per // 16], i16)
            scores = outp.tile([128, e_per // 128], f32)
            nc.sync.dma_start(srcw[:], srcw_d[:])
            nc.sync.dma_start(dstw[:], dstw_d[:])
            q = 0
            for start, tl in _edge_tiles(e_per) * reps:
                nchunk = tl // 128
                hu = gpool.tile([128, nchunk, D_FEAT], f32, tag="hu")
                hv = gpool.tile([128, nchunk, D_FEAT], f32, tag="hv")
                for dst_t, idx_t in ((hu, srcw), (hv, dstw)):
                    nc.gpsimd.dma_gather(
                        dst_t[:],
                        h_d[:],
                        idx_t[:, start // 16 : (start + tl) // 16],
                        num_idxs=tl,
                        num_idxs_reg=tl,
                        elem_size=D_FEAT,
                        single_packet=False,
                        queue_num=q % 4,
                    )
                    q += 1
                prod = ppool.tile([128, nchunk, D_FEAT], f32)
                nc.vector.tensor_mul(prod[:], hu[:], hv[:])
                nc.vector.tensor_reduce(
                    scores[:, start // 128 : start // 128 + nchunk],
                    prod[:],
                    axis=mybir.AxisListType.X,
                    op=mybir.AluOpType.add,
                )
            nc.sync.dma_start(out_d[:], scores[:])
    nc.compile()
    _BUILT[key] = nc
    return nc


def wrap_idx(ix):
    w = ix.astype(np.int16).reshape(-1, 16).T
    return np.ascontiguousarray(np.tile(w, (8, 1)))


def _kernel_flat(h, src, dst):
    nc = build_flat(E_PER)
    in_maps = []
    for k in range(N_CORES):
        sl = slice(k * E_PER, (k + 1) * E_PER)
        in_maps.append({"h": h, "srcw": wrap_idx(src[sl]), "dstw": wrap_idx(dst[sl])})
    res = run_bass_kernel_spmd(nc, in_maps, list(range(N_CORES)))
    parts = []
    for k in range(N_CORES):
        sc = res.results[k]["scores"]
        parts.append(sc.T.reshape(-1))
    return np.concatenate(parts).astype(np.float32).reshape(N_EDGES, 1)


# revision 25
# speedup vs baseline: 80.3155x; 1.5325x over previous
"""Trainium2 Bass kernel for per-edge dot products (DGL u_dot_v).

score[e] = sum_d h[src[e], d] * h[dst[e], d]   for 640K edges, 10K nodes, D=128.

Strategy (8 NeuronCores, SPMD, one-hot matmul gather — no per-edge DMA):
  - Nodes split into 79 blocks of 128. Edges bucketed by
    (src block, dst block). Src blocks ("rows") are sharded across cores
    (<=10 rows/core); each core runs a STATIC cartesian schedule of
    790 tiles: tile t covers bucket (local row t//79, dst block t%79),
    holding up to 128 edges. ~1% of buckets exceed 128 edges; overflow
    goes to 16 fixed "spill" tiles whose h blocks ship as data.
  - Per tile, the edge membership is encoded host-side as two fp8 one-hot
    matrices [128 nodes x 128 edges]. The gathers are PE matmuls:
    hu = onehot_src.T @ h_row (stationary fp8 one-hot, moving bf16 h
    block resident in SBUF), hv likewise with the full-h replica.
    A fused DVE tensor_tensor_reduce computes the 128 dot products.
  - Program structure is input-independent: all data-dependence lives in
    the shipped arrays, so one compile serves any input (SPMD-uniform).
"""

import sys

import numpy as np

for _p in ("/opt/trn_rl_repo", "/opt/pypackages"):
    if _p not in sys.path:
        sys.path.append(_p)

import ml_dtypes  # noqa: E402

import concourse.mybir as mybir  # noqa: E402
import concourse.tile as tile  # noqa: E402
from concourse import bacc, bass  # noqa: E402
from concourse.bass_utils import run_bass_kernel_spmd  # noqa: E402

N_NODES = 10000
D_FEAT = 128
N_EDGES = 640000
N_CORES = 8
P = 128
NBLK = (N_NODES + P - 1) // P  # 79 node blocks
NR = 10  # src rows per core (7 cores x 10 + 1 core x 9)
TBASE = NR * NBLK  # 790 schedule tiles
SPILL = 18  # spill tiles for buckets > 128 edges
T_TILES = TBASE + SPILL  # 808
CH = 64  # tiles per one-hot DMA chunk (1 MiB per side)
G = 4  # tiles per PSUM group (gather batch / reduce-matmul granularity)
NG = T_TILES // G  # 202 groups
DRAIN = 3  # groups per PSUM score tile (partitions 0/32/64; quadrant 3 is unusable)
ND = (NG + DRAIN - 1) // DRAIN  # drains
LAG = 4  # groups between DVE mult and PE reduce (software pipelining)

_F8 = ml_dtypes.float8_e4m3
_BF16 = ml_dtypes.bfloat16

_BUILT = {}


def build_v4(reps=1, unroll=1, mode="full"):
    """Compile the SPMD per-core program (cached; input-independent).

    v6 pipeline per 4-tile group:
      PE:  huT[d,e] = h_i @ ohs   hvT[d,e] = h_j @ ohd   (transposed gathers,
           stationary = bf16 h block, moving = fp8 one-hot)
      ACT: hvT batch-copy PSUM -> SBUF bf16
      DVE: prodT = huT(PSUM) * hvT(SBUF)  -> SBUF bf16   (one op per group)
      PE:  scores[1, G*128] = ones.T @ prodT  (reduce over features; emitted
           LAG groups late so PE never stalls on DVE; lands at a 32-aligned
           PSUM partition, 4 groups per score tile)
      DVE: drain score tile PSUM -> SBUF;  DMA out per drain.
    """
    key = ("v6", reps, unroll, mode)
    if key in _BUILT:
        return _BUILT[key]
    assert reps == 1 or reps % unroll == 0

    f32 = mybir.dt.float32
    bf16 = mybir.dt.bfloat16
    f8 = mybir.dt.float8e4

    nc = bacc.Bacc("TRN2", target_bir_lowering=False, debug=False)

    ND4 = (ND + 3) // 4
    hfull_d = nc.dram_tensor("hfull", [P, NBLK, D_FEAT], bf16, kind="ExternalInput")
    hsrc_d = nc.dram_tensor("hsrc", [P, NR, D_FEAT], bf16, kind="ExternalInput")
    spb_d = nc.dram_tensor("spb", [P, 2, SPILL, D_FEAT], bf16, kind="ExternalInput")
    ohb_d = nc.dram_tensor("ohb", [P, T_TILES, 2, P], f8, kind="ExternalInput")
    out_d = nc.dram_tensor("scores", [ND4, P, 4, G * P], f32, kind="ExternalOutput")

    with tile.TileContext(nc) as tc:
        with (
            tc.tile_pool(name="const", bufs=1) as constp,
            tc.tile_pool(name="oh", bufs=3) as ohp,
            tc.tile_pool(name="work", bufs=LAG + 2) as workp,
            tc.tile_pool(name="gps", bufs=3, space=bass.MemorySpace.PSUM) as gpsp,
            tc.tile_pool(name="rps", bufs=2, space=bass.MemorySpace.PSUM) as rpsp,
            tc.tile_pool(name="outp", bufs=3) as outp,
        ):
            hfull = constp.tile([P, NBLK, D_FEAT], bf16)
            hsrc = constp.tile([P, NR, D_FEAT], bf16)
            spb = constp.tile([P, 2, SPILL, D_FEAT], bf16)
            ones = constp.tile([P, 1], bf16)
            nc.vector.memset(ones[:], 1.0)

            def tile_operands(t):
                if t < TBASE:
                    return hsrc[:, t // NBLK, :], hfull[:, t % NBLK, :]
                return spb[:, 0, t - TBASE, :], spb[:, 1, t - TBASE, :]

            def body():
                nc.sync.dma_start(hfull[:], hfull_d[:])
                nc.scalar.dma_start(hsrc[:], hsrc_d[:])
                nc.scalar.dma_start(spb[:], spb_d[:])

                state = {"red": None, "scs": None, "pending": []}

                def flush_one():
                    g, prodT = state["pending"].pop(0)
                    q = g % DRAIN
                    if q == 0:
                        red = rpsp.tile([P, G * P], f32, tag="red")
                        state["red"] = red
                    red = state["red"]
                    nc.tensor.matmul(
                        red[32 * q : 32 * q + 1, :],
                        ones[:],
                        prodT[:].rearrange("p g e -> p (g e)"),
                    )
                    if q == DRAIN - 1 or g == NG - 1:
                        d = g // DRAIN
                        if d % 4 == 0:
                            scs = outp.tile([P, 4, G * P], f32, tag="scs")
                            state["scs"] = scs
                        scs = state["scs"]
                        if d % 2 == 0:
                            nc.scalar.activation(
                                scs[:, d % 4, :],
                                red[:],
                                mybir.ActivationFunctionType.Copy,
                            )
                        else:
                            nc.vector.tensor_copy(scs[:, d % 4, :], red[:])
                        if d % 4 == 3 or g == NG - 1:
                            eng = nc.sync if (d // 4) % 2 == 0 else nc.scalar
                            eng.dma_start(out_d[d // 4], scs[:])

                for ci, c0 in enumerate(range(0, T_TILES, CH)):
                    n = min(CH, T_TILES - c0)
                    ohb_sb = ohp.tile([P, CH, 2, P], f8, tag="ohb")
                    eng = nc.sync if ci % 2 == 0 else nc.scalar
                    eng.dma_start(ohb_sb[:, :n, :, :], ohb_d[:, c0 : c0 + n, :, :])
                    for g0 in range(0, n, G):
                        huT = gpsp.tile([P, G, P], f32, tag="huT")
                        hvT = gpsp.tile([P, G, P], f32, tag="hvT")
                        for k in range(G):
                            tt = g0 + k
                            rs, rd = tile_operands(c0 + tt)
                            nc.tensor.matmul(huT[:, k, :], rs, ohb_sb[:, tt, 0, :])
                            nc.tensor.matmul(hvT[:, k, :], rd, ohb_sb[:, tt, 1, :])
                        if mode == "gather":
                            continue
                        hvs = workp.tile([P, G, P], bf16, tag="hvs")
                        nc.scalar.activation(
                            hvs[:], hvT[:], mybir.ActivationFunctionType.Copy
                        )
                        prodT = workp.tile([P, G, P], bf16, tag="prodT")
                        nc.vector.tensor_tensor(
                            prodT[:], huT[:], hvs[:], op=mybir.AluOpType.mult
                        )
                        if mode == "nored":
                            continue
                        state["pending"].append(((c0 + g0) // G, prodT))
                        if len(state["pending"]) > LAG:
                            flush_one()
                while state["pending"]:
                    flush_one()

            if reps == 1:
                body()
            else:
                with tc.For_i(0, reps // unroll):
                    for _ in range(unroll):
                        body()

    nc.compile()
    _BUILT[key] = nc
    return nc


def prep_v4(h, src, dst):
    """Host-side packing. Returns (in_maps, core, t, p) or None if the
    spill capacity is exceeded (caller falls back)."""
    E = len(src)
    i = src // P
    j = dst // P
    core = np.minimum(i // NR, N_CORES - 1)
    m = i - core * NR
    t = m * NBLK + j

    key = core * T_TILES + t
    order = np.argsort(key, kind="stable")
    ks = key[order]
    starts = np.r_[0, np.flatnonzero(np.diff(ks)) + 1]
    sizes = np.diff(np.r_[starts, len(ks)])
    rank = np.arange(E) - np.repeat(starts, sizes)
    p = np.empty(E, np.int64)
    p[order] = rank

    # spill: edges ranked >=128 in their bucket get spill tiles, one per
    # (core, bucket, 128-chunk), numbered per core in bucket order.
    ov = p >= P
    if np.any(ov):
        sk = key[ov] * 64 + (p[ov] - P) // P  # unique per spill tile
        uniq, inv = np.unique(sk, return_inverse=True)
        ucore = (uniq // 64) // T_TILES
        # spill index within core
        sidx = np.zeros(len(uniq), np.int64)
        for c in range(N_CORES):
            sel = ucore == c
            if sel.sum() > SPILL:
                return None
            sidx[sel] = np.arange(sel.sum())
        t = t.copy()
        t[ov] = TBASE + sidx[inv]
        p = p.copy()
        p[ov] = (p[ov] - P) % P

    # h layouts
    hpad = np.zeros((NBLK * P, D_FEAT), np.float32)
    hpad[:N_NODES] = h
    h_re = np.ascontiguousarray(
        hpad.reshape(NBLK, P, D_FEAT).transpose(1, 0, 2)
    ).astype(_BF16)  # [128, 79, 128]

    # one-hots: fp8 1.0 == 0x38 bit pattern; src at [...,0,:], dst at [...,1,:]
    ohb = np.zeros((N_CORES, P, T_TILES, 2, P), np.uint8)
    ohb[core, src % P, t, 0, p] = 0x38
    ohb[core, dst % P, t, 1, p] = 0x38

    in_maps = []
    for c in range(N_CORES):
        rows = h_re[:, c * NR : c * NR + NR, :]
        if rows.shape[1] < NR:
            rows = np.concatenate(
                [rows, np.zeros((P, NR - rows.shape[1], D_FEAT), _BF16)], axis=1
            )
        spb = np.zeros((P, 2, SPILL, D_FEAT), _BF16)
        csel = np.flatnonzero((core == c) & (t >= TBASE))
        if len(csel):
            s_ids = t[csel] - TBASE
            spb[:, 0, s_ids, :] = h_re[:, i[csel], :]
            spb[:, 1, s_ids, :] = h_re[:, j[csel], :]
        in_maps.append(
            {
                "hfull": h_re,
                "hsrc": np.ascontiguousarray(rows),
                "spb": spb,
                "ohb": ohb[c].view(_F8),
            }
        )
    return in_maps, core, t, p


def kernel(h, src, dst):
    h = np.asarray(h, dtype=np.float32)
    src = np.asarray(src).astype(np.int64)
    dst = np.asarray(dst).astype(np.int64)

    prep = prep_v4(h, src, dst)
    if prep is None:
        return _kernel_flat(h, src, dst)
    in_maps, core, t, p = prep

    nc = build_v4(reps=1)
    res = run_bass_kernel_spmd(nc, in_maps, list(range(N_CORES)))
    stacked = np.stack([np.asarray(res.results[c]["scores"]) for c in range(N_CORES)])
    g = t // G  # [8, ND4, 128, 4, G*128]
    d = g // DRAIN
    out = stacked[
        core, d // 4, 32 * (g % DRAIN), d % 4, (t % G) * P + p
    ].astype(np.float32)
    return out.reshape(N_EDGES, 1)


# ---------------------------------------------------------------------------
# Fallback: SWDGE dma_gather path (slow but fully general) — used only if
# spill capacity is exceeded (pathological edge distributions).
# ---------------------------------------------------------------------------

TILE_E = 4096
E_PER = N_EDGES // N_CORES


def _edge_tiles(e_per):
    tiles = []
    s = 0
    while s < e_per:
        tl = min(TILE_E, e_per - s)
        tiles.append((s, tl))
        s += tl
    return tiles


def build_flat(e_per, reps=1):
    key = ("flat", e_per, reps)
    if key in _BUILT:
        return _BUILT[key]
    i16 = mybir.dt.int16
    f32 = mybir.dt.float32
    nc = bacc.Bacc("TRN2", target_bir_lowering=False, debug=False, num_swdge_queues=4)
    h_d = nc.dram_tensor("h", [N_NODES, D_FEAT], f32, kind="ExternalInput")
    srcw_d = nc.dram_tensor("srcw", [128, e_per // 16], i16, kind="ExternalInput")
    dstw_d = nc.dram_tensor("dstw", [128, e_per // 16], i16, kind="ExternalInput")
    out_d = nc.dram_tensor("scores", [128, e_per // 128], f32, kind="ExternalOutput")
    with tile.TileContext(nc) as tc:
        with (
            tc.tile_pool(name="const", bufs=1) as constp,
            tc.tile_pool(name="gather", bufs=3) as gpool,
            tc.tile_pool(name="prod", bufs=2) as ppool,
            tc.tile_pool(name="outp", bufs=1) as outp,
        ):
            srcw = constp.tile([128, e_per // 16], i16)
            dstw = constp.tile([128, e_per // 16], i16)
            scores = outp.tile([128, e_per // 128], f32)
            nc.sync.dma_start(srcw[:], srcw_d[:])
            nc.sync.dma_start(dstw[:], dstw_d[:])
            q = 0
            for start, tl in _edge_tiles(e_per) * reps:
                nchunk = tl // 128
                hu = gpool.tile([128, nchunk, D_FEAT], f32, tag="hu")
                hv = gpool.tile([128, nchunk, D_FEAT], f32, tag="hv")
                for dst_t, idx_t in ((hu, srcw), (hv, dstw)):
                    nc.gpsimd.dma_gather(
                        dst_t[:],
                        h_d[:],
                        idx_t[:, start // 16 : (start + tl) // 16],
                        num_idxs=tl,
                        num_idxs_reg=tl,
                        elem_size=D_FEAT,
                        single_packet=False,
                        queue_num=q % 4,
                    )
                    q += 1
                prod = ppool.tile([128, nchunk, D_FEAT], f32)
                nc.vector.tensor_mul(prod[:], hu[:], hv[:])
                nc.vector.tensor_reduce(
                    scores[:, start // 128 : start // 128 + nchunk],
                    prod[:],
                    axis=mybir.AxisListType.X,
                    op=mybir.AluOpType.add,
                )
            nc.sync.dma_start(out_d[:], scores[:])
    nc.compile()
    _BUILT[key] = nc
    return nc


def wrap_idx(ix):
    w = ix.astype(np.int16).reshape(-1, 16).T
    return np.ascontiguousarray(np.tile(w, (8, 1)))


def _kernel_flat(h, src, dst):
    nc = build_flat(E_PER)
    in_maps = []
    for k in range(N_CORES):
        sl = slice(k * E_PER, (k + 1) * E_PER)
        in_maps.append({"h": h, "srcw": wrap_idx(src[sl]), "dstw": wrap_idx(dst[sl])})
    res = run_bass_kernel_spmd(nc, in_maps, list(range(N_CORES)))
    parts = []
    for k in range(N_CORES):
        sc = res.results[k]["scores"]
        parts.append(sc.T.reshape(-1))
    return np.concatenate(parts).astype(np.float32).reshape(N_EDGES, 1)


# revision 31
# speedup vs baseline: 150.0657x; 1.8685x over previous
"""Trainium2 Bass kernel for per-edge dot products (DGL u_dot_v).

score[e] = sum_d h[src[e], d] * h[dst[e], d]   for 640K edges, 10K nodes, D=128.

Strategy (8 NeuronCores, SPMD, one-hot matmul gather — no per-edge DMA):
  - Nodes split into 79 blocks of 128. Edges bucketed by
    (src block, dst block). Src blocks ("rows") are sharded across cores
    (<=10 rows/core); each core runs a STATIC cartesian schedule of
    790 tiles: tile t covers bucket (local row t//79, dst block t%79),
    holding up to 128 edges. ~1% of buckets exceed 128 edges; overflow
    goes to 16 fixed "spill" tiles whose h blocks ship as data.
  - Per tile, the edge membership is encoded host-side as two fp8 one-hot
    matrices [128 nodes x 128 edges]. The gathers are PE matmuls:
    hu = onehot_src.T @ h_row (stationary fp8 one-hot, moving bf16 h
    block resident in SBUF), hv likewise with the full-h replica.
    A fused DVE tensor_tensor_reduce computes the 128 dot products.
  - Program structure is input-independent: all data-dependence lives in
    the shipped arrays, so one compile serves any input (SPMD-uniform).
"""

import sys

import numpy as np

for _p in ("/opt/trn_rl_repo", "/opt/pypackages"):
    if _p not in sys.path:
        sys.path.append(_p)

import ml_dtypes  # noqa: E402

import concourse.mybir as mybir  # noqa: E402
import concourse.tile as tile  # noqa: E402
from concourse import bacc, bass  # noqa: E402
from concourse.bass_utils import run_bass_kernel_spmd  # noqa: E402

N_NODES = 10000
D_FEAT = 128
N_EDGES = 640000
N_CORES = 8
P = 128
NBLK = (N_NODES + P - 1) // P  # 79 node blocks
NR = 10  # src rows per core (7 cores x 10 + 1 core x 9)
TBASE = NR * NBLK  # 790 schedule tiles
SPILL = 18  # spill tiles for buckets > 128 edges
T_TILES = TBASE + SPILL  # 808
CH = 64  # tiles per one-hot DMA chunk (1 MiB per side)
G = 4  # tiles per PSUM group (gather batch / reduce-matmul granularity)
NG = T_TILES // G  # 202 groups
DRAIN = 3  # groups per PSUM score tile (partitions 0/32/64; quadrant 3 is unusable)
ND = (NG + DRAIN - 1) // DRAIN  # drains
LAG = 4  # groups between DVE mult and PE reduce (software pipelining)

_F8 = ml_dtypes.float8_e4m3
_BF16 = ml_dtypes.bfloat16

_BUILT = {}


def build_v4(reps=1, unroll=1, mode="full"):
    """Compile the SPMD per-core program (cached; input-independent).

    v6 pipeline per 4-tile group:
      PE:  huT[d,e] = h_i @ ohs   hvT[d,e] = h_j @ ohd   (transposed gathers,
           stationary = bf16 h block, moving = fp8 one-hot)
      ACT: hvT batch-copy PSUM -> SBUF bf16
      DVE: prodT = huT(PSUM) * hvT(SBUF)  -> SBUF bf16   (one op per group)
      PE:  scores[1, G*128] = ones.T @ prodT  (reduce over features; emitted
           LAG groups late so PE never stalls on DVE; lands at a 32-aligned
           PSUM partition, 4 groups per score tile)
      DVE: drain score tile PSUM -> SBUF;  DMA out per drain.
    """
    key = ("v6", reps, unroll, mode)
    if key in _BUILT:
        return _BUILT[key]
    assert reps == 1 or reps % unroll == 0

    f32 = mybir.dt.float32
    bf16 = mybir.dt.bfloat16
    f8 = mybir.dt.float8e4

    nc = bacc.Bacc("TRN2", target_bir_lowering=False, debug=False)

    ND4 = (ND + 3) // 4
    hfull_d = nc.dram_tensor("hfull", [P, NBLK, D_FEAT], bf16, kind="ExternalInput")
    hsrc_d = nc.dram_tensor("hsrc", [P, NR, D_FEAT], bf16, kind="ExternalInput")
    spb_d = nc.dram_tensor("spb", [P, 2, SPILL, D_FEAT], bf16, kind="ExternalInput")
    # per group g: [:, g, 0, k, :] = src one-hot of tile 4g+k (quad-contiguous
    # for the batched src gather), [:, g, 1, k, :] = dst one-hot.
    ohb_d = nc.dram_tensor("ohb", [P, NG, 2, G, P], f8, kind="ExternalInput")
    out_d = nc.dram_tensor("scores", [ND4, P, 4, G * P], f32, kind="ExternalOutput")

    with tile.TileContext(nc) as tc:
        with (
            tc.tile_pool(name="const", bufs=1) as constp,
            tc.tile_pool(name="oh", bufs=3) as ohp,
            tc.tile_pool(name="work", bufs=LAG + 2) as workp,
            tc.tile_pool(name="gps", bufs=3, space=bass.MemorySpace.PSUM) as gpsp,
            tc.tile_pool(name="rps", bufs=2, space=bass.MemorySpace.PSUM) as rpsp,
            tc.tile_pool(name="outp", bufs=3) as outp,
        ):
            hfull = constp.tile([P, NBLK, D_FEAT], bf16)
            hsrc = constp.tile([P, NR, D_FEAT], bf16)
            spb = constp.tile([P, 2, SPILL, D_FEAT], bf16)
            ones = constp.tile([P, 1], bf16)
            nc.vector.memset(ones[:], 1.0)

            def tile_operands(t):
                if t < TBASE:
                    return hsrc[:, t // NBLK, :], hfull[:, t % NBLK, :]
                return spb[:, 0, t - TBASE, :], spb[:, 1, t - TBASE, :]

            def body():
                nc.sync.dma_start(hfull[:], hfull_d[:])
                nc.sync.dma_start(hsrc[:], hsrc_d[:])
                nc.sync.dma_start(spb[:], spb_d[:])

                state = {"red": None, "scs": None, "pending": []}

                def flush_one():
                    g, prodT = state["pending"].pop(0)
                    q = g % DRAIN
                    if q == 0:
                        red = rpsp.tile([P, G * P], f32, tag="red")
                        state["red"] = red
                    red = state["red"]
                    nc.tensor.matmul(
                        red[32 * q : 32 * q + 1, :],
                        ones[:],
                        prodT[:].rearrange("p g e -> p (g e)"),
                    )
                    if q == DRAIN - 1 or g == NG - 1:
                        d = g // DRAIN
                        if d % 4 == 0:
                            scs = outp.tile([P, 4, G * P], f32, tag="scs")
                            state["scs"] = scs
                        scs = state["scs"]
                        nc.scalar.activation(
                            scs[:, d % 4, :],
                            red[:],
                            mybir.ActivationFunctionType.Copy,
                        )
                        if d % 4 == 3 or g == NG - 1:
                            nc.sync.dma_start(out_d[d // 4], scs[:])

                CHG = CH // G  # groups per chunk
                for ci, c0 in enumerate(range(0, NG, CHG)):
                    n = min(CHG, NG - c0)
                    ohb_sb = ohp.tile([P, CHG, 2, G, P], f8, tag="ohb")
                    nc.sync.dma_start(
                        ohb_sb[:, :n, :, :, :], ohb_d[:, c0 : c0 + n, :, :, :]
                    )
                    for gg in range(n):
                        g = c0 + gg
                        t0 = g * G
                        huT = gpsp.tile([P, G, P], f32, tag="huT")
                        hvT = gpsp.tile([P, G, P], f32, tag="hvT")
                        ops = [tile_operands(t0 + k) for k in range(G)]
                        same_src = t0 + G <= TBASE and (
                            t0 // NBLK == (t0 + G - 1) // NBLK
                        )
                        if same_src:
                            # whole group shares the src row block: one matmul
                            nc.tensor.matmul(
                                huT[:].rearrange("p g e -> p (g e)"),
                                ops[0][0],
                                ohb_sb[:, gg, 0, :, :].rearrange("p g e -> p (g e)"),
                            )
                        else:
                            for k in range(G):
                                nc.tensor.matmul(
                                    huT[:, k, :], ops[k][0], ohb_sb[:, gg, 0, k, :]
                                )
                        for k in range(G):
                            nc.tensor.matmul(
                                hvT[:, k, :], ops[k][1], ohb_sb[:, gg, 1, k, :]
                            )
                        if mode == "gather":
                            continue
                        hvs = workp.tile([P, G, P], bf16, tag="hvs")
                        nc.scalar.activation(
                            hvs[:], hvT[:], mybir.ActivationFunctionType.Copy
                        )
                        prodT = workp.tile([P, G, P], bf16, tag="prodT")
                        nc.vector.tensor_tensor(
                            prodT[:], huT[:], hvs[:], op=mybir.AluOpType.mult
                        )
                        if mode == "nored":
                            continue
                        state["pending"].append((g, prodT))
                        if len(state["pending"]) > LAG:
                            flush_one()
                while state["pending"]:
                    flush_one()

            if reps == 1:
                body()
            else:
                with tc.For_i(0, reps // unroll):
                    for _ in range(unroll):
                        body()

    nc.compile()
    _BUILT[key] = nc
    return nc


def prep_v4(h, src, dst):
    """Host-side packing. Returns (in_maps, core, t, p) or None if the
    spill capacity is exceeded (caller falls back)."""
    E = len(src)
    i = src // P
    j = dst // P
    core = np.minimum(i // NR, N_CORES - 1)
    m = i - core * NR
    t = m * NBLK + j

    key = core * T_TILES + t
    order = np.argsort(key, kind="stable")
    ks = key[order]
    starts = np.r_[0, np.flatnonzero(np.diff(ks)) + 1]
    sizes = np.diff(np.r_[starts, len(ks)])
    rank = np.arange(E) - np.repeat(starts, sizes)
    p = np.empty(E, np.int64)
    p[order] = rank

    # spill: edges ranked >=128 in their bucket get spill tiles, one per
    # (core, bucket, 128-chunk), numbered per core in bucket order.
    ov = p >= P
    if np.any(ov):
        sk = key[ov] * 64 + (p[ov] - P) // P  # unique per spill tile
        uniq, inv = np.unique(sk, return_inverse=True)
        ucore = (uniq // 64) // T_TILES
        # spill index within core
        sidx = np.zeros(len(uniq), np.int64)
        for c in range(N_CORES):
            sel = ucore == c
            if sel.sum() > SPILL:
                return None
            sidx[sel] = np.arange(sel.sum())
        t = t.copy()
        t[ov] = TBASE + sidx[inv]
        p = p.copy()
        p[ov] = (p[ov] - P) % P

    # h layouts
    hpad = np.zeros((NBLK * P, D_FEAT), np.float32)
    hpad[:N_NODES] = h
    h_re = np.ascontiguousarray(
        hpad.reshape(NBLK, P, D_FEAT).transpose(1, 0, 2)
    ).astype(_BF16)  # [128, 79, 128]

    # one-hots: fp8 1.0 == 0x38 bit pattern; group-major layout with the four
    # src one-hots of a group contiguous (for the batched src gather matmul)
    ohb = np.zeros((N_CORES, P, NG, 2, G, P), np.uint8)
    ohb[core, src % P, t // G, 0, t % G, p] = 0x38
    ohb[core, dst % P, t // G, 1, t % G, p] = 0x38

    in_maps = []
    for c in range(N_CORES):
        rows = h_re[:, c * NR : c * NR + NR, :]
        if rows.shape[1] < NR:
            rows = np.concatenate(
                [rows, np.zeros((P, NR - rows.shape[1], D_FEAT), _BF16)], axis=1
            )
        spb = np.zeros((P, 2, SPILL, D_FEAT), _BF16)
        csel = np.flatnonzero((core == c) & (t >= TBASE))
        if len(csel):
            s_ids = t[csel] - TBASE
            spb[:, 0, s_ids, :] = h_re[:, i[csel], :]
            spb[:, 1, s_ids, :] = h_re[:, j[csel], :]
        in_maps.append(
            {
                "hfull": h_re,
                "hsrc": np.ascontiguousarray(rows),
                "spb": spb,
                "ohb": ohb[c].view(_F8),
            }
        )
    return in_maps, core, t, p


def kernel(h, src, dst):
    h = np.asarray(h, dtype=np.float32)
    src = np.asarray(src).astype(np.int64)
    dst = np.asarray(dst).astype(np.int64)

    prep = prep_v4(h, src, dst)
    if prep is None:
        return _kernel_flat(h, src, dst)
    in_maps, core, t, p = prep

    nc = build_v4(reps=1)
    res = run_bass_kernel_spmd(nc, in_maps, list(range(N_CORES)))
    stacked = np.stack([np.asarray(res.results[c]["scores"]) for c in range(N_CORES)])
    g = t // G  # [8, ND4, 128, 4, G*128]
    d = g // DRAIN
    out = stacked[
        core, d // 4, 32 * (g % DRAIN), d % 4, (t % G) * P + p
    ].astype(np.float32)
    return out.reshape(N_EDGES, 1)


# ---------------------------------------------------------------------------
# Fallback: SWDGE dma_gather path (slow but fully general) — used only if
# spill capacity is exceeded (pathological edge distributions).
# ---------------------------------------------------------------------------

TILE_E = 4096
E_PER = N_EDGES // N_CORES


def _edge_tiles(e_per):
    tiles = []
    s = 0
    while s < e_per:
        tl = min(TILE_E, e_per - s)
        tiles.append((s, tl))
        s += tl
    return tiles


def build_flat(e_per, reps=1):
    key = ("flat", e_per, reps)
    if key in _BUILT:
        return _BUILT[key]
    i16 = mybir.dt.int16
    f32 = mybir.dt.float32
    nc = bacc.Bacc("TRN2", target_bir_lowering=False, debug=False, num_swdge_queues=4)
    h_d = nc.dram_tensor("h", [N_NODES, D_FEAT], f32, kind="ExternalInput")
    srcw_d = nc.dram_tensor("srcw", [128, e_per // 16], i16, kind="ExternalInput")
    dstw_d = nc.dram_tensor("dstw", [128, e_per // 16], i16, kind="ExternalInput")
    out_d = nc.dram_tensor("scores", [128, e_per // 128], f32, kind="ExternalOutput")
    with tile.TileContext(nc) as tc:
        with (
            tc.tile_pool(name="const", bufs=1) as constp,
            tc.tile_pool(name="gather", bufs=3) as gpool,
            tc.tile_pool(name="prod", bufs=2) as ppool,
            tc.tile_pool(name="outp", bufs=1) as outp,
        ):
            srcw = constp.tile([128, e_per // 16], i16)
            dstw = constp.tile([128, e_per // 16], i16)
            scores = outp.tile([128, e_per // 128], f32)
            nc.sync.dma_start(srcw[:], srcw_d[:])
            nc.sync.dma_start(dstw[:], dstw_d[:])
            q = 0
            for start, tl in _edge_tiles(e_per) * reps:
                nchunk = tl // 128
                hu = gpool.tile([128, nchunk, D_FEAT], f32, tag="hu")
                hv = gpool.tile([128, nchunk, D_FEAT], f32, tag="hv")
                for dst_t, idx_t in ((hu, srcw), (hv, dstw)):
                    nc.gpsimd.dma_gather(
                        dst_t[:],
                        h_d[:],
                        idx_t[:, start // 16 : (start + tl) // 16],
                        num_idxs=tl,
                        num_idxs_reg=tl,
                        elem_size=D_FEAT,
                        single_packet=False,
                        queue_num=q % 4,
                    )
                    q += 1
                prod = ppool.tile([128, nchunk, D_FEAT], f32)
                nc.vector.tensor_mul(prod[:], hu[:], hv[:])
                nc.vector.tensor_reduce(
                    scores[:, start // 128 : start // 128 + nchunk],
                    prod[:],
                    axis=mybir.AxisListType.X,
                    op=mybir.AluOpType.add,
                )
            nc.sync.dma_start(out_d[:], scores[:])
    nc.compile()
    _BUILT[key] = nc
    return nc


def wrap_idx(ix):
    w = ix.astype(np.int16).reshape(-1, 16).T
    return np.ascontiguousarray(np.tile(w, (8, 1)))


def _kernel_flat(h, src, dst):
    nc = build_flat(E_PER)
    in_maps = []
    for k in range(N_CORES):
        sl = slice(k * E_PER, (k + 1) * E_PER)
        in_maps.append({"h": h, "srcw": wrap_idx(src[sl]), "dstw": wrap_idx(dst[sl])})
    res = run_bass_kernel_spmd(nc, in_maps, list(range(N_CORES)))
    parts = []
    for k in range(N_CORES):
        sc = res.results[k]["scores"]
        parts.append(sc.T.reshape(-1))
    return np.concatenate(parts).astype(np.float32).reshape(N_EDGES, 1)


# revision 32
# speedup vs baseline: 163.1903x; 1.0875x over previous
"""Trainium2 Bass kernel for per-edge dot products (DGL u_dot_v).

score[e] = sum_d h[src[e], d] * h[dst[e], d]   for 640K edges, 10K nodes, D=128.

Strategy (8 NeuronCores, SPMD, one-hot matmul gather — no per-edge DMA):
  - Nodes split into 79 blocks of 128. Edges bucketed by
    (src block, dst block). Src blocks ("rows") are sharded across cores
    (<=10 rows/core); each core runs a STATIC cartesian schedule of
    790 tiles: tile t covers bucket (local row t//79, dst block t%79),
    holding up to 128 edges. ~1% of buckets exceed 128 edges; overflow
    goes to 16 fixed "spill" tiles whose h blocks ship as data.
  - Per tile, the edge membership is encoded host-side as two fp8 one-hot
    matrices [128 nodes x 128 edges]. Per 4-tile group the device runs:
    PE transposed gathers huT/hvT[d,e] (stationary bf16 h block, moving
    fp8 one-hot; the four src gathers of a group batch into ONE matmul
    since they share the row block), ScalarE batch-copy of hvT to SBUF
    bf16 (DVE may read only one PSUM operand), one DVE elementwise mult,
    and a PE ones-matmul reducing over features (emitted LAG groups late
    so PE never stalls; lands at 32-aligned PSUM partitions, 3 groups
    per score tile, drained by ScalarE + DMA).
  - Program structure is input-independent: all data-dependence lives in
    the shipped arrays, so one compile serves any input (SPMD-uniform).
  - Engine instruction streams are kept small (PE ~2.5K instrs) because
    NX IRAM is 128KB/engine; oversized loop bodies thrash instruction
    fetch (measured 5x slowdown at 4x body size).
"""

import sys

import numpy as np

for _p in ("/opt/trn_rl_repo", "/opt/pypackages"):
    if _p not in sys.path:
        sys.path.append(_p)

import ml_dtypes  # noqa: E402

import concourse.mybir as mybir  # noqa: E402
import concourse.tile as tile  # noqa: E402
from concourse import bacc, bass  # noqa: E402
from concourse.bass_utils import run_bass_kernel_spmd  # noqa: E402

N_NODES = 10000
D_FEAT = 128
N_EDGES = 640000
N_CORES = 8
P = 128
NBLK = (N_NODES + P - 1) // P  # 79 node blocks
NR = 10  # src rows per core (7 cores x 10 + 1 core x 9)
TBASE = NR * NBLK  # 790 schedule tiles
SPILL = 18  # spill tiles for buckets > 128 edges
T_TILES = TBASE + SPILL  # 808
CH = 64  # tiles per one-hot DMA chunk (1 MiB per side)
G = 4  # tiles per PSUM group (gather batch / reduce-matmul granularity)
NG = T_TILES // G  # 202 groups
DRAIN = 3  # groups per PSUM score tile (partitions 0/32/64; quadrant 3 is unusable)
ND = (NG + DRAIN - 1) // DRAIN  # drains
LAG = 4  # groups between DVE mult and PE reduce (software pipelining)

_F8 = ml_dtypes.float8_e4m3
_BF16 = ml_dtypes.bfloat16

_BUILT = {}


def build_v4(reps=1, unroll=1, mode="full"):
    """Compile the SPMD per-core program (cached; input-independent).

    v6 pipeline per 4-tile group:
      PE:  huT[d,e] = h_i @ ohs   hvT[d,e] = h_j @ ohd   (transposed gathers,
           stationary = bf16 h block, moving = fp8 one-hot)
      ACT: hvT batch-copy PSUM -> SBUF bf16
      DVE: prodT = huT(PSUM) * hvT(SBUF)  -> SBUF bf16   (one op per group)
      PE:  scores[1, G*128] = ones.T @ prodT  (reduce over features; emitted
           LAG groups late so PE never stalls on DVE; lands at a 32-aligned
           PSUM partition, 4 groups per score tile)
      DVE: drain score tile PSUM -> SBUF;  DMA out per drain.
    """
    key = ("v6", reps, unroll, mode)
    if key in _BUILT:
        return _BUILT[key]
    assert reps == 1 or reps % unroll == 0

    f32 = mybir.dt.float32
    bf16 = mybir.dt.bfloat16
    f8 = mybir.dt.float8e4

    nc = bacc.Bacc("TRN2", target_bir_lowering=False, debug=False)

    ND4 = (ND + 3) // 4
    hfull_d = nc.dram_tensor("hfull", [P, NBLK, D_FEAT], bf16, kind="ExternalInput")
    hsrc_d = nc.dram_tensor("hsrc", [P, NR, D_FEAT], bf16, kind="ExternalInput")
    spb_d = nc.dram_tensor("spb", [P, 2, SPILL, D_FEAT], bf16, kind="ExternalInput")
    # per group g: [:, g, 0, k, :] = src one-hot of tile 4g+k (quad-contiguous
    # for the batched src gather), [:, g, 1, k, :] = dst one-hot.
    ohb_d = nc.dram_tensor("ohb", [P, NG, 2, G, P], f8, kind="ExternalInput")
    out_d = nc.dram_tensor("scores", [ND4, P, 4, G * P], f32, kind="ExternalOutput")

    with tile.TileContext(nc) as tc:
        with (
            tc.tile_pool(name="const", bufs=1) as constp,
            tc.tile_pool(name="oh", bufs=3) as ohp,
            tc.tile_pool(name="work", bufs=LAG + 2) as workp,
            tc.tile_pool(name="gps", bufs=3, space=bass.MemorySpace.PSUM) as gpsp,
            tc.tile_pool(name="rps", bufs=2, space=bass.MemorySpace.PSUM) as rpsp,
            tc.tile_pool(name="outp", bufs=3) as outp,
        ):
            hfull = constp.tile([P, NBLK, D_FEAT], bf16)
            hsrc = constp.tile([P, NR, D_FEAT], bf16)
            spb = constp.tile([P, 2, SPILL, D_FEAT], bf16)
            ones = constp.tile([P, 1], bf16)
            nc.vector.memset(ones[:], 1.0)

            def tile_operands(t):
                if t < TBASE:
                    return hsrc[:, t // NBLK, :], hfull[:, t % NBLK, :]
                return spb[:, 0, t - TBASE, :], spb[:, 1, t - TBASE, :]

            def body():
                nc.sync.dma_start(hfull[:], hfull_d[:])
                nc.sync.dma_start(hsrc[:], hsrc_d[:])
                nc.sync.dma_start(spb[:], spb_d[:])

                state = {"red": None, "scs": None, "pending": []}

                def flush_one():
                    g, prodT = state["pending"].pop(0)
                    q = g % DRAIN
                    if q == 0:
                        red = rpsp.tile([P, G * P], f32, tag="red")
                        state["red"] = red
                    red = state["red"]
                    nc.tensor.matmul(
                        red[32 * q : 32 * q + 1, :],
                        ones[:],
                        prodT[:].rearrange("p g e -> p (g e)"),
                    )
                    if q == DRAIN - 1 or g == NG - 1:
                        d = g // DRAIN
                        if d % 4 == 0:
                            scs = outp.tile([P, 4, G * P], f32, tag="scs")
                            state["scs"] = scs
                        scs = state["scs"]
                        nc.scalar.activation(
                            scs[:, d % 4, :],
                            red[:],
                            mybir.ActivationFunctionType.Copy,
                        )
                        if d % 4 == 3 or g == NG - 1:
                            nc.sync.dma_start(out_d[d // 4], scs[:])

                CHG = CH // G  # groups per chunk
                for ci, c0 in enumerate(range(0, NG, CHG)):
                    n = min(CHG, NG - c0)
                    ohb_sb = ohp.tile([P, CHG, 2, G, P], f8, tag="ohb")
                    nc.sync.dma_start(
                        ohb_sb[:, :n, :, :, :], ohb_d[:, c0 : c0 + n, :, :, :]
                    )
                    for gg in range(n):
                        g = c0 + gg
                        t0 = g * G
                        huT = gpsp.tile([P, G, P], f32, tag="huT")
                        hvT = gpsp.tile([P, G, P], f32, tag="hvT")
                        ops = [tile_operands(t0 + k) for k in range(G)]
                        same_src = t0 + G <= TBASE and (
                            t0 // NBLK == (t0 + G - 1) // NBLK
                        )
                        if same_src:
                            # whole group shares the src row block: one matmul
                            nc.tensor.matmul(
                                huT[:].rearrange("p g e -> p (g e)"),
                                ops[0][0],
                                ohb_sb[:, gg, 0, :, :].rearrange("p g e -> p (g e)"),
                            )
                        else:
                            for k in range(G):
                                nc.tensor.matmul(
                                    huT[:, k, :], ops[k][0], ohb_sb[:, gg, 0, k, :]
                                )
                        for k in range(G):
                            nc.tensor.matmul(
                                hvT[:, k, :], ops[k][1], ohb_sb[:, gg, 1, k, :]
                            )
                        if mode == "gather":
                            continue
                        hvs = workp.tile([P, G, P], bf16, tag="hvs")
                        nc.scalar.activation(
                            hvs[:], hvT[:], mybir.ActivationFunctionType.Copy
                        )
                        prodT = workp.tile([P, G, P], bf16, tag="prodT")
                        nc.vector.tensor_tensor(
                            prodT[:], huT[:], hvs[:], op=mybir.AluOpType.mult
                        )
                        if mode == "nored":
                            continue
                        state["pending"].append((g, prodT))
                        if len(state["pending"]) > LAG:
                            flush_one()
                while state["pending"]:
                    flush_one()

            if reps == 1:
                body()
            else:
                with tc.For_i(0, reps // unroll):
                    for _ in range(unroll):
                        body()

    nc.compile()
    _BUILT[key] = nc
    return nc


def prep_v4(h, src, dst):
    """Host-side packing. Returns (in_maps, core, t, p) or None if the
    spill capacity is exceeded (caller falls back)."""
    E = len(src)
    i = src // P
    j = dst // P
    core = np.minimum(i // NR, N_CORES - 1)
    m = i - core * NR
    t = m * NBLK + j

    key = core * T_TILES + t
    order = np.argsort(key, kind="stable")
    ks = key[order]
    starts = np.r_[0, np.flatnonzero(np.diff(ks)) + 1]
    sizes = np.diff(np.r_[starts, len(ks)])
    rank = np.arange(E) - np.repeat(starts, sizes)
    p = np.empty(E, np.int64)
    p[order] = rank

    # spill: edges ranked >=128 in their bucket get spill tiles, one per
    # (core, bucket, 128-chunk), numbered per core in bucket order.
    ov = p >= P
    if np.any(ov):
        sk = key[ov] * 64 + (p[ov] - P) // P  # unique per spill tile
        uniq, inv = np.unique(sk, return_inverse=True)
        ucore = (uniq // 64) // T_TILES
        # spill index within core
        sidx = np.zeros(len(uniq), np.int64)
        for c in range(N_CORES):
            sel = ucore == c
            if sel.sum() > SPILL:
                return None
            sidx[sel] = np.arange(sel.sum())
        t = t.copy()
        t[ov] = TBASE + sidx[inv]
        p = p.copy()
        p[ov] = (p[ov] - P) % P

    # h layouts
    hpad = np.zeros((NBLK * P, D_FEAT), np.float32)
    hpad[:N_NODES] = h
    h_re = np.ascontiguousarray(
        hpad.reshape(NBLK, P, D_FEAT).transpose(1, 0, 2)
    ).astype(_BF16)  # [128, 79, 128]

    # one-hots: fp8 1.0 == 0x38 bit pattern; group-major layout with the four
    # src one-hots of a group contiguous (for the batched src gather matmul)
    ohb = np.zeros((N_CORES, P, NG, 2, G, P), np.uint8)
    ohb[core, src % P, t // G, 0, t % G, p] = 0x38
    ohb[core, dst % P, t // G, 1, t % G, p] = 0x38

    in_maps = []
    for c in range(N_CORES):
        rows = h_re[:, c * NR : c * NR + NR, :]
        if rows.shape[1] < NR:
            rows = np.concatenate(
                [rows, np.zeros((P, NR - rows.shape[1], D_FEAT), _BF16)], axis=1
            )
        spb = np.zeros((P, 2, SPILL, D_FEAT), _BF16)
        csel = np.flatnonzero((core == c) & (t >= TBASE))
        if len(csel):
            s_ids = t[csel] - TBASE
            spb[:, 0, s_ids, :] = h_re[:, i[csel], :]
            spb[:, 1, s_ids, :] = h_re[:, j[csel], :]
        in_maps.append(
            {
                "hfull": h_re,
                "hsrc": np.ascontiguousarray(rows),
                "spb": spb,
                "ohb": ohb[c].view(_F8),
            }
        )
    return in_maps, core, t, p


def kernel(h, src, dst):
    h = np.asarray(h, dtype=np.float32)
    src = np.asarray(src).astype(np.int64)
    dst = np.asarray(dst).astype(np.int64)

    prep = prep_v4(h, src, dst)
    if prep is None:
        return _kernel_flat(h, src, dst)
    in_maps, core, t, p = prep

    nc = build_v4(reps=1)
    res = run_bass_kernel_spmd(nc, in_maps, list(range(N_CORES)))
    stacked = np.stack([np.asarray(res.results[c]["scores"]) for c in range(N_CORES)])
    g = t // G  # [8, ND4, 128, 4, G*128]
    d = g // DRAIN
    out = stacked[
        core, d // 4, 32 * (g % DRAIN), d % 4, (t % G) * P + p
    ].astype(np.float32)
    return out.reshape(N_EDGES, 1)


# ---------------------------------------------------------------------------
# Fallback: SWDGE dma_gather path (slow but fully general) — used only if
# spill capacity is exceeded (pathological edge distributions).
# ---------------------------------------------------------------------------

TILE_E = 4096
E_PER = N_EDGES // N_CORES


def _edge_tiles(e_per):
    tiles = []
    s = 0
    while s < e_per:
        tl = min(TILE_E, e_per - s)
        tiles.append((s, tl))
        s += tl
    return tiles


def build_flat(e_per, reps=1):
    key = ("flat", e_per, reps)
    if key in _BUILT:
        return _BUILT[key]
    i16 = mybir.dt.int16
    f32 = mybir.dt.float32
    nc = bacc.Bacc("TRN2", target_bir_lowering=False, debug=False, num_swdge_queues=4)
    h_d = nc.dram_tensor("h", [N_NODES, D_FEAT], f32, kind="ExternalInput")
    srcw_d = nc.dram_tensor("srcw", [128, e_per // 16], i16, kind="ExternalInput")
    dstw_d = nc.dram_tensor("dstw", [128, e_per // 16], i16, kind="ExternalInput")
    out_d = nc.dram_tensor("scores", [128, e_per // 128], f32, kind="ExternalOutput")
    with tile.TileContext(nc) as tc:
        with (
            tc.tile_pool(name="const", bufs=1) as constp,
            tc.tile_pool(name="gather", bufs=3) as gpool,
            tc.tile_pool(name="prod", bufs=2) as ppool,
            tc.tile_pool(name="outp", bufs=1) as outp,
        ):
            srcw = constp.tile([128, e_per // 16], i16)
            dstw = constp.tile([128, e_per // 16], i16)
            scores = outp.tile([128, e_per // 128], f32)
            nc.sync.dma_start(srcw[:], srcw_d[:])
            nc.sync.dma_start(dstw[:], dstw_d[:])
            q = 0
            for start, tl in _edge_tiles(e_per) * reps:
                nchunk = tl // 128
                hu = gpool.tile([128, nchunk, D_FEAT], f32, tag="hu")
                hv = gpool.tile([128, nchunk, D_FEAT], f32, tag="hv")
                for dst_t, idx_t in ((hu, srcw), (hv, dstw)):
                    nc.gpsimd.dma_gather(
                        dst_t[:],
                        h_d[:],
                        idx_t[:, start // 16 : (start + tl) // 16],
                        num_idxs=tl,
                        num_idxs_reg=tl,
                        elem_size=D_FEAT,
                        single_packet=False,
                        queue_num=q % 4,
                    )
                    q += 1
                prod = ppool.tile([128, nchunk, D_FEAT], f32)
                nc.vector.tensor_mul(prod[:], hu[:], hv[:])
                nc.vector.tensor_reduce(
                    scores[:, start // 128 : start // 128 + nchunk],
                    prod[:],
                    axis=mybir.AxisListType.X,
                    op=mybir.AluOpType.add,
                )
            nc.sync.dma_start(out_d[:], scores[:])
    nc.compile()
    _BUILT[key] = nc
    return nc


def wrap_idx(ix):
    w = ix.astype(np.int16).reshape(-1, 16).T
    return np.ascontiguousarray(np.tile(w, (8, 1)))


def _kernel_flat(h, src, dst):
    nc = build_flat(E_PER)
    in_maps = []
    for k in range(N_CORES):
        sl = slice(k * E_PER, (k + 1) * E_PER)
        in_maps.append({"h": h, "srcw": wrap_idx(src[sl]), "dstw": wrap_idx(dst[sl])})
    res = run_bass_kernel_spmd(nc, in_maps, list(range(N_CORES)))
    parts = []
    for k in range(N_CORES):
        sc = res.results[k]["scores"]
        parts.append(sc.T.reshape(-1))
    return np.concatenate(parts).astype(np.float32).reshape(N_EDGES, 1)


# revision 34
# speedup vs baseline: 171.3181x; 1.0498x over previous
"""Trainium2 Bass kernel for per-edge dot products (DGL u_dot_v).

score[e] = sum_d h[src[e], d] * h[dst[e], d]   for 640K edges, 10K nodes, D=128.

Strategy (8 NeuronCores, SPMD, one-hot matmul gather — no per-edge DMA):
  - Nodes split into 79 blocks of 128. Edges bucketed by
    (src block, dst block). Src blocks ("rows") are sharded across cores
    (<=10 rows/core); each core runs a STATIC cartesian schedule of
    790 tiles: tile t covers bucket (local row t//79, dst block t%79),
    holding up to 128 edges. ~1% of buckets exceed 128 edges; overflow
    goes to 16 fixed "spill" tiles whose h blocks ship as data.
  - Per tile, the edge membership is encoded host-side as two fp8 one-hot
    matrices [128 nodes x 128 edges]. Per 4-tile group the device runs:
    PE transposed gathers huT/hvT[d,e] (stationary bf16 h block, moving
    fp8 one-hot; the four src gathers of a group batch into ONE matmul
    since they share the row block), ScalarE batch-copy of hvT to SBUF
    bf16 (DVE may read only one PSUM operand), one DVE elementwise mult,
    and a PE ones-matmul reducing over features (emitted LAG groups late
    so PE never stalls; lands at 32-aligned PSUM partitions, 3 groups
    per score tile, drained by ScalarE + DMA).
  - Program structure is input-independent: all data-dependence lives in
    the shipped arrays, so one compile serves any input (SPMD-uniform).
  - Engine instruction streams are kept small (PE ~2.5K instrs) because
    NX IRAM is 128KB/engine; oversized loop bodies thrash instruction
    fetch (measured 5x slowdown at 4x body size).
"""

import sys

import numpy as np

for _p in ("/opt/trn_rl_repo", "/opt/pypackages"):
    if _p not in sys.path:
        sys.path.append(_p)

import ml_dtypes  # noqa: E402

import concourse.mybir as mybir  # noqa: E402
import concourse.tile as tile  # noqa: E402
from concourse import bacc, bass  # noqa: E402
from concourse.bass_utils import run_bass_kernel_spmd  # noqa: E402

N_NODES = 10000
D_FEAT = 128
N_EDGES = 640000
N_CORES = 8
P = 128
NBLK = (N_NODES + P - 1) // P  # 79 node blocks
NR = 10  # src rows per core (7 cores x 10 + 1 core x 9)
TBASE = NR * NBLK  # 790 schedule tiles
SPILL = 18  # spill tiles for buckets > 128 edges
T_TILES = TBASE + SPILL  # 808
CH = 64  # tiles per one-hot DMA chunk (2 MiB merged)
G = 4  # tiles per PSUM group (gather batch / reduce-matmul granularity)
NG = T_TILES // G  # 202 groups
DRAIN = 3  # groups per PSUM score tile (partitions 0/32/64; quadrant 3 is unusable)
ND = (NG + DRAIN - 1) // DRAIN  # drains
LAG = 4  # groups between DVE mult and PE reduce (software pipelining)

_F8 = ml_dtypes.float8_e4m3
_BF16 = ml_dtypes.bfloat16

_BUILT = {}


def build_v4(reps=1, unroll=1, mode="full"):
    """Compile the SPMD per-core program (cached; input-independent).

    v6 pipeline per 4-tile group:
      PE:  huT[d,e] = h_i @ ohs   hvT[d,e] = h_j @ ohd   (transposed gathers,
           stationary = bf16 h block, moving = fp8 one-hot)
      ACT: hvT batch-copy PSUM -> SBUF bf16
      DVE: prodT = huT(PSUM) * hvT(SBUF)  -> SBUF bf16   (one op per group)
      PE:  scores[1, G*128] = ones.T @ prodT  (reduce over features; emitted
           LAG groups late so PE never stalls on DVE; lands at a 32-aligned
           PSUM partition, 4 groups per score tile)
      DVE: drain score tile PSUM -> SBUF;  DMA out per drain.
    """
    key = ("v6", reps, unroll, mode)
    if key in _BUILT:
        return _BUILT[key]
    assert reps == 1 or reps % unroll == 0

    f32 = mybir.dt.float32
    bf16 = mybir.dt.bfloat16
    f8 = mybir.dt.float8e4

    nc = bacc.Bacc("TRN2", target_bir_lowering=False, debug=False)

    ND4 = (ND + 3) // 4
    hfull_d = nc.dram_tensor("hfull", [P, NBLK, D_FEAT], bf16, kind="ExternalInput")
    hsrc_d = nc.dram_tensor("hsrc", [P, NR, D_FEAT], bf16, kind="ExternalInput")
    spb_d = nc.dram_tensor("spb", [P, 2, SPILL, D_FEAT], bf16, kind="ExternalInput")
    # per group g: [:, g, 0, k, :] = src one-hot of tile 4g+k (quad-contiguous
    # for the batched src gather), [:, g, 1, k, :] = dst one-hot.
    ohb_d = nc.dram_tensor("ohb", [P, NG, 2, G, P], f8, kind="ExternalInput")
    out_d = nc.dram_tensor("scores", [ND4, P, 4, G * P], f32, kind="ExternalOutput")

    with tile.TileContext(nc) as tc:
        with (
            tc.tile_pool(name="const", bufs=1) as constp,
            tc.tile_pool(name="oh", bufs=3) as ohp,
            tc.tile_pool(name="work", bufs=LAG + 2) as workp,
            tc.tile_pool(name="gps", bufs=3, space=bass.MemorySpace.PSUM) as gpsp,
            tc.tile_pool(name="rps", bufs=2, space=bass.MemorySpace.PSUM) as rpsp,
            tc.tile_pool(name="outp", bufs=3) as outp,
        ):
            hfull = constp.tile([P, NBLK, D_FEAT], bf16)
            hsrc = constp.tile([P, NR, D_FEAT], bf16)
            spb = constp.tile([P, 2, SPILL, D_FEAT], bf16)
            ones = constp.tile([P, 1], bf16)
            nc.vector.memset(ones[:], 1.0)

            def tile_operands(t):
                if t < TBASE:
                    return hsrc[:, t // NBLK, :], hfull[:, t % NBLK, :]
                return spb[:, 0, t - TBASE, :], spb[:, 1, t - TBASE, :]

            def body():
                nc.sync.dma_start(hfull[:], hfull_d[:])
                nc.sync.dma_start(hsrc[:], hsrc_d[:])
                nc.sync.dma_start(spb[:], spb_d[:])

                state = {"red": None, "scs": None, "pending": []}

                def flush_one():
                    g, prodT = state["pending"].pop(0)
                    q = g % DRAIN
                    if q == 0:
                        red = rpsp.tile([P, G * P], f32, tag="red")
                        state["red"] = red
                    red = state["red"]
                    nc.tensor.matmul(
                        red[32 * q : 32 * q + 1, :],
                        ones[:],
                        prodT[:].rearrange("p g e -> p (g e)"),
                    )
                    if q == DRAIN - 1 or g == NG - 1:
                        d = g // DRAIN
                        if d % 4 == 0:
                            scs = outp.tile([P, 4, G * P], f32, tag="scs")
                            state["scs"] = scs
                        scs = state["scs"]
                        nc.scalar.activation(
                            scs[:, d % 4, :],
                            red[:],
                            mybir.ActivationFunctionType.Copy,
                        )
                        if d % 4 == 3 or g == NG - 1:
                            nc.sync.dma_start(out_d[d // 4], scs[:])

                CHG = CH // G  # groups per chunk
                for ci, c0 in enumerate(range(0, NG, CHG)):
                    n = min(CHG, NG - c0)
                    ohb_sb = ohp.tile([P, CHG, 2, G, P], f8, tag="ohb")
                    nc.sync.dma_start(
                        ohb_sb[:, :n, :, :, :], ohb_d[:, c0 : c0 + n, :, :, :]
                    )
                    for gg in range(n):
                        g = c0 + gg
                        t0 = g * G
                        huT = gpsp.tile([P, G, P], f32, tag="huT")
                        hvT = gpsp.tile([P, G, P], f32, tag="hvT")
                        ops = [tile_operands(t0 + k) for k in range(G)]
                        same_src = t0 + G <= TBASE and (
                            t0 // NBLK == (t0 + G - 1) // NBLK
                        )
                        if same_src:
                            # whole group shares the src row block: one matmul
                            nc.tensor.matmul(
                                huT[:].rearrange("p g e -> p (g e)"),
                                ops[0][0],
                                ohb_sb[:, gg, 0, :, :].rearrange("p g e -> p (g e)"),
                            )
                        else:
                            for k in range(G):
                                nc.tensor.matmul(
                                    huT[:, k, :], ops[k][0], ohb_sb[:, gg, 0, k, :]
                                )
                        for k in range(G):
                            nc.tensor.matmul(
                                hvT[:, k, :], ops[k][1], ohb_sb[:, gg, 1, k, :]
                            )
                        if mode == "gather":
                            continue
                        hvs = workp.tile([P, G, P], bf16, tag="hvs")
                        nc.scalar.activation(
                            hvs[:], hvT[:], mybir.ActivationFunctionType.Copy
                        )
                        prodT = workp.tile([P, G, P], bf16, tag="prodT")
                        nc.vector.tensor_tensor(
                            prodT[:], huT[:], hvs[:], op=mybir.AluOpType.mult
                        )
                        if mode == "nored":
                            continue
                        state["pending"].append((g, prodT))
                        if len(state["pending"]) > LAG:
                            flush_one()
                while state["pending"]:
                    flush_one()

            if reps == 1:
                body()
            else:
                with tc.For_i(0, reps // unroll):
                    for _ in range(unroll):
                        body()

    nc.compile()
    _BUILT[key] = nc
    return nc


def prep_v4(h, src, dst):
    """Host-side packing. Returns (in_maps, core, t, p) or None if the
    spill capacity is exceeded (caller falls back)."""
    E = len(src)
    i = src // P
    j = dst // P
    core = np.minimum(i // NR, N_CORES - 1)
    m = i - core * NR
    t = m * NBLK + j

    key = core * T_TILES + t
    order = np.argsort(key, kind="stable")
    ks = key[order]
    starts = np.r_[0, np.flatnonzero(np.diff(ks)) + 1]
    sizes = np.diff(np.r_[starts, len(ks)])
    rank = np.arange(E) - np.repeat(starts, sizes)
    p = np.empty(E, np.int64)
    p[order] = rank

    # spill: edges ranked >=128 in their bucket get spill tiles, one per
    # (core, bucket, 128-chunk), numbered per core in bucket order.
    ov = p >= P
    if np.any(ov):
        sk = key[ov] * 64 + (p[ov] - P) // P  # unique per spill tile
        uniq, inv = np.unique(sk, return_inverse=True)
        ucore = (uniq // 64) // T_TILES
        # spill index within core
        sidx = np.zeros(len(uniq), np.int64)
        for c in range(N_CORES):
            sel = ucore == c
            if sel.sum() > SPILL:
                return None
            sidx[sel] = np.arange(sel.sum())
        t = t.copy()
        t[ov] = TBASE + sidx[inv]
        p = p.copy()
        p[ov] = (p[ov] - P) % P

    # h layouts
    hpad = np.zeros((NBLK * P, D_FEAT), np.float32)
    hpad[:N_NODES] = h
    h_re = np.ascontiguousarray(
        hpad.reshape(NBLK, P, D_FEAT).transpose(1, 0, 2)
    ).astype(_BF16)  # [128, 79, 128]

    # one-hots: fp8 1.0 == 0x38 bit pattern; group-major layout with the four
    # src one-hots of a group contiguous (for the batched src gather matmul)
    ohb = np.zeros((N_CORES, P, NG, 2, G, P), np.uint8)
    ohb[core, src % P, t // G, 0, t % G, p] = 0x38
    ohb[core, dst % P, t // G, 1, t % G, p] = 0x38

    in_maps = []
    for c in range(N_CORES):
        rows = h_re[:, c * NR : c * NR + NR, :]
        if rows.shape[1] < NR:
            rows = np.concatenate(
                [rows, np.zeros((P, NR - rows.shape[1], D_FEAT), _BF16)], axis=1
            )
        spb = np.zeros((P, 2, SPILL, D_FEAT), _BF16)
        csel = np.flatnonzero((core == c) & (t >= TBASE))
        if len(csel):
            s_ids = t[csel] - TBASE
            spb[:, 0, s_ids, :] = h_re[:, i[csel], :]
            spb[:, 1, s_ids, :] = h_re[:, j[csel], :]
        in_maps.append(
            {
                "hfull": h_re,
                "hsrc": np.ascontiguousarray(rows),
                "spb": spb,
                "ohb": ohb[c].view(_F8),
            }
        )
    return in_maps, core, t, p


def kernel(h, src, dst):
    h = np.asarray(h, dtype=np.float32)
    src = np.asarray(src).astype(np.int64)
    dst = np.asarray(dst).astype(np.int64)

    prep = prep_v4(h, src, dst)
    if prep is None:
        return _kernel_flat(h, src, dst)
    in_maps, core, t, p = prep

    nc = build_v4(reps=1)
    res = run_bass_kernel_spmd(nc, in_maps, list(range(N_CORES)))
    stacked = np.stack([np.asarray(res.results[c]["scores"]) for c in range(N_CORES)])
    g = t // G  # [8, ND4, 128, 4, G*128]
    d = g // DRAIN
    out = stacked[
        core, d // 4, 32 * (g % DRAIN), d % 4, (t % G) * P + p
    ].astype(np.float32)
    return out.reshape(N_EDGES, 1)


# ---------------------------------------------------------------------------
# Fallback: SWDGE dma_gather path (slow but fully general) — used only if
# spill capacity is exceeded (pathological edge distributions).
# ---------------------------------------------------------------------------

TILE_E = 4096
E_PER = N_EDGES // N_CORES


def _edge_tiles(e_per):
    tiles = []
    s = 0
    while s < e_per:
        tl = min(TILE_E, e_per - s)
        tiles.append((s, tl))
        s += tl
    return tiles


def build_flat(e_per, reps=1):
    key = ("flat", e_per, reps)
    if key in _BUILT:
        return _BUILT[key]
    i16 = mybir.dt.int16
    f32 = mybir.dt.float32
    nc = bacc.Bacc("TRN2", target_bir_lowering=False, debug=False, num_swdge_queues=4)
    h_d = nc.dram_tensor("h", [N_NODES, D_FEAT], f32, kind="ExternalInput")
    srcw_d = nc.dram_tensor("srcw", [128, e_per // 16], i16, kind="ExternalInput")
    dstw_d = nc.dram_tensor("dstw", [128, e_per // 16], i16, kind="ExternalInput")
    out_d = nc.dram_tensor("scores", [128, e_per // 128], f32, kind="ExternalOutput")
    with tile.TileContext(nc) as tc:
        with (
            tc.tile_pool(name="const", bufs=1) as constp,
            tc.tile_pool(name="gather", bufs=3) as gpool,
            tc.tile_pool(name="prod", bufs=2) as ppool,
            tc.tile_pool(name="outp", bufs=1) as outp,
        ):
            srcw = constp.tile([128, e_per // 16], i16)
            dstw = constp.tile([128, e_per // 16], i16)
            scores = outp.tile([128, e_per // 128], f32)
            nc.sync.dma_start(srcw[:], srcw_d[:])
            nc.sync.dma_start(dstw[:], dstw_d[:])
            q = 0
            for start, tl in _edge_tiles(e_per) * reps:
                nchunk = tl // 128
                hu = gpool.tile([128, nchunk, D_FEAT], f32, tag="hu")
                hv = gpool.tile([128, nchunk, D_FEAT], f32, tag="hv")
                for dst_t, idx_t in ((hu, srcw), (hv, dstw)):
                    nc.gpsimd.dma_gather(
                        dst_t[:],
                        h_d[:],
                        idx_t[:, start // 16 : (start + tl) // 16],
                        num_idxs=tl,
                        num_idxs_reg=tl,
                        elem_size=D_FEAT,
                        single_packet=False,
                        queue_num=q % 4,
                    )
                    q += 1
                prod = ppool.tile([128, nchunk, D_FEAT], f32)
                nc.vector.tensor_mul(prod[:], hu[:], hv[:])
                nc.vector.tensor_reduce(
                    scores[:, start // 128 : start // 128 + nchunk],
                    prod[:],
                    axis=mybir.AxisListType.X,
                    op=mybir.AluOpType.add,
                )
            nc.sync.dma_start(out_d[:], scores[:])
    nc.compile()
    _BUILT[key] = nc
    return nc


def wrap_idx(ix):
    w = ix.astype(np.int16).reshape(-1, 16).T
    return np.ascontiguousarray(np.tile(w, (8, 1)))


def _kernel_flat(h, src, dst):
    nc = build_flat(E_PER)
    in_maps = []
    for k in range(N_CORES):
        sl = slice(k * E_PER, (k + 1) * E_PER)
        in_maps.append({"h": h, "srcw": wrap_idx(src[sl]), "dstw": wrap_idx(dst[sl])})
    res = run_bass_kernel_spmd(nc, in_maps, list(range(N_CORES)))
    parts = []
    for k in range(N_CORES):
        sc = res.results[k]["scores"]
        parts.append(sc.T.reshape(-1))
    return np.concatenate(parts).astype(np.float32).reshape(N_EDGES, 1)
